# revision 1
# baseline (speedup 1.0000x reference)
"""Trainium2 Bass kernel for nn_CompressedMoE_31550829757014.

The reference's router/top-k computation is dead code -- the output is just
    out = x @ expert_w[0].T + expert_b[0]
i.e. one (8192 x 2048) x (2048 x 2048) GEMM with a bias.

Strategy:
  * Data-parallel over tokens: 8192 tokens / 8 cores = 1024 tokens per core.
  * Host-side prep: transpose x-shard and W0 so the contraction dim (d) lands
    on SBUF partitions, and split both operands into bf16 hi/lo pairs.
    3-term split GEMM (hi*hi + hi*lo + lo*hi, fp32 PSUM accumulation) gives
    ~1e-5 relative error at 3x bf16 cost -- far faster than native fp32
    matmul (4 passes) on the PE.
  * Device: W0T hi/lo resident in SBUF (16 MB), x streamed per 128-token
    tile, dense back-to-back matmuls (PE stays warm), bias fused into the
    PSUM->SBUF copyback on the vector engine.
"""

import numpy as np
import ml_dtypes

BF16 = ml_dtypes.bfloat16

B, S, D, E = 4, 2048, 2048, 8
N_CORES = 8
T_CORE = (B * S) // N_CORES  # 1024 tokens per core


def _build_nc(T, DD, O, n_tile=512, mode="split3", mm_dtype_name="bfloat16"):
    """Build the per-core Bass program: out[T,O] = xT.T @ w0T + bias.

    DRAM params (per core):
      xh, xl : [T/128, 128, DD/128, 128]  x-shard transposed + pre-tiled
               ([m,p,k,t] = xT[k*128+p, m*128+t]), hi/lo split
      wh, wl : [DD, O]  W0.T, hi/lo split (replicated across cores)
      bias   : [128, O] f32  b0 broadcast to 128 partitions (replicated)
      out    : [T, O]   f32

    mode="split3": psum += xh@wh + xh@wl + xl@wh (3-term split GEMM,
                   rel err ~4e-6 vs fp32)
    mode="single": psum += xh@wh only (xl/wl absent; used to probe dtypes --
                   float32r measured 156 us but rel err 1.2e-4)
    """
    import concourse.bacc as bacc
    import concourse.mybir as mybir
    import concourse.tile as tile
    from concourse.bass import ts

    P = 128
    KT = DD // P          # k tiles
    MT = T // P           # token tiles
    NT = O // n_tile      # output-feature tiles

    nc = bacc.Bacc(
        "TRN2", target_bir_lowering=False, debug=False, num_devices=N_CORES
    )
    f32 = mybir.dt.float32
    bf16 = getattr(mybir.dt, mm_dtype_name)

    # x is fed pre-tiled: [MT, P, KT, P] with [m, p, k, t] = xT[k*P+p, m*P+t],
    # so each m-tile's load is 128 partitions x 4KB contiguous.
    single = mode == "single"
    xh = nc.declare_dram_parameter("xh", [MT, P, KT, P], bf16, isOutput=False)
    wh = nc.declare_dram_parameter("wh", [DD, O], bf16, isOutput=False)
    if not single:
        xl = nc.declare_dram_parameter("xl", [MT, P, KT, P], bf16, isOutput=False)
        wl = nc.declare_dram_parameter("wl", [DD, O], bf16, isOutput=False)
        wl_r = wl.rearrange("(k p) o -> p k o", p=P)
    bias = nc.declare_dram_parameter("bias", [P, O], f32, isOutput=False)
    out = nc.declare_dram_parameter("out", [T, O], f32, isOutput=True)

    wh_r = wh.rearrange("(k p) o -> p k o", p=P)

    with tile.TileContext(nc) as tc:
        with (
            tc.tile_pool(name="wpool", bufs=1) as wpool,
            tc.tile_pool(name="xpool", bufs=4) as xpool,
            tc.tile_pool(name="opool", bufs=8) as opool,
            tc.tile_pool(name="psum", bufs=8, space="PSUM") as psum,
        ):
            x_tiles = {}

            def load_xh(m):
                xh_t = xpool.tile([P, KT, P], bf16, tag="xh", name=f"xh_{m}")
                nc.sync.dma_start(xh_t[:], xh[m])
                x_tiles[m] = (xh_t, None)

            def load_xl(m):
                if single:
                    return
                xl_t = xpool.tile([P, KT, P], bf16, tag="xl", name=f"xl_{m}")
                nc.sync.dma_start(xl_t[:], xl[m])
                x_tiles[m] = (x_tiles[m][0], xl_t)

            def load_x(m):
                load_xh(m)
                load_xl(m)

            # Resident weights, one tile per k-slice so matmuls only wait on
            # the k-slices they actually read. Emission order = DMA issue
            # order: the first matmul needs only xh[0] + wh k=0, so those go
            # first; the rest of W streams in k (use) order underneath the
            # compute.
            wh_sb = [None] * KT
            wl_sb = [None] * KT

            def load_wh(k):
                th = wpool.tile([P, O], bf16, tag=f"wh{k}", name=f"wh_sb{k}")
                nc.sync.dma_start(th[:], wh_r[:, k])
                wh_sb[k] = th

            def load_wl(k):
                if single:
                    return
                tl = wpool.tile([P, O], bf16, tag=f"wl{k}", name=f"wl_sb{k}")
                nc.sync.dma_start(tl[:], wl_r[:, k])
                wl_sb[k] = tl

            def load_bias():
                b = wpool.tile([P, O], f32, tag="bias")
                nc.sync.dma_start(b[:], bias[:])
                return b

            # Emission order tunes DMA issue order: the first matmul needs
            # only xh[m=0] + wh k=0, so exactly those go first. When m=0's
            # wl pass is deferred (defer_w), the whole wh stream goes before
            # any wl so m=0 is paced by wh arrival alone.
            defer_w = mode != "single" and MT >= 3
            bias_sb = None
            load_xh(0)
            load_wh(0)
            load_xl(0)
            if not defer_w:
                load_wl(0)
            for k in range(1, KT):
                load_wh(k)
                if not defer_w:
                    load_wl(k)
                if k == KT // 2 and MT > 1:
                    load_x(1)
                if k == (KT * 5) // 8 and not defer_w:
                    bias_sb = load_bias()
            if defer_w:
                for k in range(KT):
                    load_wl(k)
                    if k == (KT * 5) // 8:
                        bias_sb = load_bias()
            if MT > 1 and KT < 2:
                load_x(1)
            if bias_sb is None:
                bias_sb = load_bias()

            # m=0's wl-dependent pass (hl) is deferred into m=1's window:
            # while m=0 runs, the DMA stream only has to deliver wh
            # (~290 GB/s demand < ~325 GB/s supply), so m=0 is no longer
            # DMA-paced; the deferred matmuls run interleaved with m=1's
            # k-loop once the wl slices have arrived.
            defer_m0 = mode != "single" and MT >= 3
            psums0 = None
            xh0_sb = None

            for m in range(MT):
                xh_sb, xl_sb = x_tiles.pop(m)

                psums = [
                    psum.tile([P, n_tile], f32, tag="ps", name=f"ps_{m}_{n}")
                    for n in range(NT)
                ]
                def emit_mms(k, ns):
                    first = k == 0
                    last = k == KT - 1
                    # Pass order hh, lh, hl: both wh passes run before the wl
                    # pass so wl[k]'s DMA gets 8 matmuls of extra slack while
                    # m=0 is still DMA-paced.
                    for n in ns:
                        nc.tensor.matmul(
                            psums[n][:], xh_sb[:, k], wh_sb[k][:, ts(n, n_tile)],
                            start=first, stop=(last and mode == "single"),
                        )
                    if mode == "single":
                        return
                    for n in ns:
                        nc.tensor.matmul(
                            psums[n][:], xl_sb[:, k], wh_sb[k][:, ts(n, n_tile)],
                            start=False, stop=False,
                        )
                    for n in ns:
                        nc.tensor.matmul(
                            psums[n][:], xh_sb[:, k], wl_sb[k][:, ts(n, n_tile)],
                            start=False, stop=last,
                        )

                def emit_copyback(n, ps=None, mi=None):
                    ps = psums if ps is None else ps
                    mi = m if mi is None else mi
                    ob = opool.tile([P, n_tile], f32, tag="ob", name=f"ob_{mi}_{n}")
                    nc.vector.tensor_add(
                        out=ob[:], in0=ps[n][:], in1=bias_sb[:, ts(n, n_tile)]
                    )
                    nc.sync.dma_start(out[ts(mi, P), ts(n, n_tile)], ob[:])

                if defer_m0 and m == 0:
                    # hh + lh passes only (wh-dependent); hl is deferred.
                    for k in range(KT):
                        for n in range(NT):
                            nc.tensor.matmul(
                                psums[n][:], xh_sb[:, k],
                                wh_sb[k][:, ts(n, n_tile)],
                                start=(k == 0), stop=False,
                            )
                        for n in range(NT):
                            nc.tensor.matmul(
                                psums[n][:], xl_sb[:, k],
                                wh_sb[k][:, ts(n, n_tile)],
                                start=False, stop=False,
                            )
                    psums0 = psums
                    xh0_sb = xh_sb
                elif defer_m0 and m == 1:
                    for k in range(KT):
                        emit_mms(k, list(range(NT)))
                        # m=0's deferred hl pass, one k-slice per m=1 k-step
                        for n in range(NT):
                            nc.tensor.matmul(
                                psums0[n][:], xh0_sb[:, k],
                                wl_sb[k][:, ts(n, n_tile)],
                                start=False, stop=(k == KT - 1),
                            )
                    for n in range(NT):
                        emit_copyback(n, ps=psums0, mi=0)
                    for n in range(NT):
                        emit_copyback(n)
                elif m == MT - 1:
                    # Last m-tile: finish one psum bank at a time so the
                    # copyback + store of bank n overlaps bank n+1's matmuls
                    # instead of all serializing after the final matmul.
                    for n in range(NT):
                        for k in range(KT):
                            emit_mms(k, [n])
                        emit_copyback(n)
                else:
                    for k in range(KT):
                        emit_mms(k, list(range(NT)))
                    for n in range(NT):
                        emit_copyback(n)

                if m + 2 < MT:
                    load_x(m + 2)

    nc.compile()
    return nc


def _split_bf16(a_f32):
    """Split fp32 array into bf16 hi + bf16 lo with x ~= hi + lo."""
    hi = a_f32.astype(BF16)
    lo = (a_f32 - hi.astype(np.float32)).astype(BF16)
    return hi, lo


def _tile_xT(xt_2d):
    """[D, T] -> [T//128, 128, D//128, 128] with [m,p,k,t] = xt[k*128+p, m*128+t]."""
    DD, T = xt_2d.shape
    return np.ascontiguousarray(
        xt_2d.reshape(DD // 128, 128, T // 128, 128).transpose(2, 1, 0, 3)
    )


def _prep_in_maps(x, expert_w, expert_b):
    x2 = np.asarray(x, dtype=np.float32).reshape(B * S, D)
    w0t = np.ascontiguousarray(np.asarray(expert_w, dtype=np.float32)[0].T)  # [D, O]
    wh, wl = _split_bf16(w0t)
    bias = np.ascontiguousarray(
        np.broadcast_to(np.asarray(expert_b, dtype=np.float32)[0], (128, D)).astype(
            np.float32
        )
    )
    in_maps = []
    for c in range(N_CORES):
        xct = x2[c * T_CORE : (c + 1) * T_CORE].T  # [D, T] view
        xh, xl = _split_bf16(xct)
        in_maps.append(
            {
                "xh": _tile_xT(xh),
                "xl": _tile_xT(xl),
                "wh": wh,
                "wl": wl,
                "bias": bias,
            }
        )
    return in_maps


_NC_CACHE = {}


def kernel(x, router_w, expert_w, expert_b):
    from concourse.bass_utils import run_bass_kernel_spmd

    in_maps = _prep_in_maps(x, expert_w, expert_b)
    if "nc" not in _NC_CACHE:
        _NC_CACHE["nc"] = _build_nc(T_CORE, D, D)
    nc = _NC_CACHE["nc"]
    res = run_bass_kernel_spmd(nc, in_maps, list(range(N_CORES)))
    outs = [res.results[c]["out"] for c in range(N_CORES)]
    full = np.concatenate(outs, axis=0).reshape(B, S, D)
    return np.ascontiguousarray(full.astype(np.float32))



# revision 2
# speedup vs baseline: 2.5739x; 2.5739x over previous
"""Trainium2 Bass kernel for nn_CompressedMoE_31550829757014.

The reference's router/top-k computation is dead code -- the output is just
    out = x @ expert_w[0].T + expert_b[0]
i.e. one (8192 x 2048) x (2048 x 2048) GEMM with a bias.

Strategy:
  * Data-parallel over tokens: 8192 tokens / 8 cores = 1024 tokens per core.
  * Single-pass bf16 matmul with fp32 PSUM accumulation. Input rounding to
    bf16 gives ~2.4e-3 relative RMS error, well inside the 2e-2 gate, at 1/3
    the PE cost of the previous 3-term hi/lo split GEMM.
  * Per-core compute roofline: 2*1024*2048*2048 FLOP / 78.6 TF/s = 109 us.
  * k-major loop over PAIRs of 128-token tiles (2 m x 4 n = 8 PSUM banks):
    the first pair's compute window (~27.6 us) covers the full 8 MB weight
    stream (~23 us at 358 GB/s), so the PE never waits on W after startup.
  * Bias is fused into the PSUM->SBUF copyback on the vector engine, emitted
    immediately after each bank's last matmul so banks free up early.
"""

import numpy as np
import ml_dtypes

BF16 = ml_dtypes.bfloat16

B, S, D, E = 4, 2048, 2048, 8
N_CORES = 8
T_CORE = (B * S) // N_CORES  # 1024 tokens per core


def _build_nc(T, DD, O, n_tile=512, pair=2):
    """Build the per-core Bass program: out[T,O] = xT.T @ w0T + bias.

    DRAM params (per core):
      xh   : [T/128, 128, DD/128, 128]  x-shard transposed + pre-tiled
             ([m,p,k,t] = xT[k*128+p, m*128+t]), bf16
      wh   : [DD, O]  W0.T bf16 (replicated across cores)
      bias : [128, O] f32  b0 broadcast to 128 partitions (replicated)
      out  : [T, O]   f32
    """
    import concourse.bacc as bacc
    import concourse.mybir as mybir
    import concourse.tile as tile
    from concourse.bass import ts

    P = 128
    KT = DD // P          # 16 contraction tiles
    MT = T // P           # 8 token tiles
    NT = O // n_tile      # 4 output-feature tiles
    PAIR = pair           # m-tiles per psum group; PAIR*NT <= 8 banks

    nc = bacc.Bacc(
        "TRN2", target_bir_lowering=False, debug=False, num_devices=N_CORES
    )
    f32 = mybir.dt.float32
    bf16 = mybir.dt.bfloat16

    xh = nc.declare_dram_parameter("xh", [MT, P, KT, P], bf16, isOutput=False)
    wh = nc.declare_dram_parameter("wh", [DD, O], bf16, isOutput=False)
    bias = nc.declare_dram_parameter("bias", [P, O], f32, isOutput=False)
    out = nc.declare_dram_parameter("out", [T, O], f32, isOutput=True)

    wh_r = wh.rearrange("(k p) o -> p k o", p=P)

    with tile.TileContext(nc) as tc:
        with (
            tc.tile_pool(name="wpool", bufs=1) as wpool,
            tc.tile_pool(name="xpool", bufs=MT) as xpool,
            tc.tile_pool(name="opool", bufs=8) as opool,
            tc.tile_pool(name="psum", bufs=8, space="PSUM") as psum,
        ):
            x_sb = {}

            def load_x(m):
                t = xpool.tile([P, KT, P], bf16, tag="x", name=f"x_{m}")
                nc.sync.dma_start(t[:], xh[m])
                x_sb[m] = t

            wh_sb = [None] * KT

            def load_wh(k):
                t = wpool.tile([P, O], bf16, tag=f"wh{k}", name=f"wh{k}")
                nc.sync.dma_start(t[:], wh_r[:, k])
                wh_sb[k] = t

            # DMA emission order = issue order. The first pair's k-step k
            # runs at ~2.9 + 1.73k us; W slice k lands at ~2.9 + 1.43(k+1) us
            # (both streams ~358 GB/s), so compute never waits on W after the
            # first couple of slices. x2/x3 slot in late in the W stream,
            # still well before the second pair needs them (~30 us); bias is
            # only read by the first copyback (~28 us).
            load_x(0)
            load_x(1)
            for k in range(13):
                load_wh(k)
            load_x(2)
            load_x(3)
            for k in range(13, KT):
                load_wh(k)
            b_sb = wpool.tile([P, O], f32, tag="bias")
            nc.sync.dma_start(b_sb[:], bias[:])
            for m in range(4, MT):
                load_x(m)

            for g in range(MT // PAIR):
                ms = list(range(g * PAIR, (g + 1) * PAIR))
                ps = {
                    (m, n): psum.tile([P, n_tile], f32, tag="ps", name=f"ps_{m}_{n}")
                    for m in ms
                    for n in range(NT)
                }
                for k in range(KT):
                    last = k == KT - 1
                    for m in ms:
                        for n in range(NT):
                            nc.tensor.matmul(
                                ps[(m, n)][:],
                                x_sb[m][:, k],
                                wh_sb[k][:, ts(n, n_tile)],
                                start=(k == 0),
                                stop=last,
                            )
                            if last:
                                # Emit the copyback right behind each bank's
                                # final matmul: the DVE add runs under the
                                # remaining matmuls and the bank is free
                                # before the next group's first matmul.
                                ob = opool.tile(
                                    [P, n_tile], f32, tag="ob", name=f"ob_{m}_{n}"
                                )
                                nc.vector.tensor_add(
                                    out=ob[:],
                                    in0=ps[(m, n)][:],
                                    in1=b_sb[:, ts(n, n_tile)],
                                )
                                nc.sync.dma_start(
                                    out[ts(m, P), ts(n, n_tile)], ob[:]
                                )

    nc.compile()
    return nc


def _tile_xT(xt_2d):
    """[D, T] -> [T//128, 128, D//128, 128] with [m,p,k,t] = xt[k*128+p, m*128+t]."""
    DD, T = xt_2d.shape
    return np.ascontiguousarray(
        xt_2d.reshape(DD // 128, 128, T // 128, 128).transpose(2, 1, 0, 3)
    )


def _prep_in_maps(x, expert_w, expert_b):
    x2 = np.asarray(x, dtype=np.float32).reshape(B * S, D)
    w0t = np.ascontiguousarray(np.asarray(expert_w, dtype=np.float32)[0].T)  # [D, O]
    wh = w0t.astype(BF16)
    bias = np.ascontiguousarray(
        np.broadcast_to(np.asarray(expert_b, dtype=np.float32)[0], (128, D)).astype(
            np.float32
        )
    )
    in_maps = []
    for c in range(N_CORES):
        xct = x2[c * T_CORE : (c + 1) * T_CORE].T  # [D, T] view
        in_maps.append(
            {
                "xh": _tile_xT(xct.astype(BF16)),
                "wh": wh,
                "bias": bias,
            }
        )
    return in_maps


_NC_CACHE = {}


def kernel(x, router_w, expert_w, expert_b):
    from concourse.bass_utils import run_bass_kernel_spmd

    in_maps = _prep_in_maps(x, expert_w, expert_b)
    if "nc" not in _NC_CACHE:
        _NC_CACHE["nc"] = _build_nc(T_CORE, D, D)
    nc = _NC_CACHE["nc"]
    res = run_bass_kernel_spmd(nc, in_maps, list(range(N_CORES)))
    outs = [res.results[c]["out"] for c in range(N_CORES)]
    full = np.concatenate(outs, axis=0).reshape(B, S, D)
    return np.ascontiguousarray(full.astype(np.float32))
